# revision 1
# baseline (speedup 1.0000x reference)
"""Trainium2 Bass kernel for nn_DegreePrediction.

Computes y[u] = sum_{s,t,v} (x*W_t)[s,t] * (W_r*r_zeros + r_const)[s,t,u,v]
with N=80, streaming the three rank-4 tensors from HBM.

Sharding: leading s axis split across 8 cores (10 s-values = 800 (s,t) rows
per core, contiguous in DRAM). Each core computes a partial y[80]; partials
are summed on the host (the output is tiny, so no device collective).

The kernel is purely HBM/DMA-bound (arithmetic intensity ~0.33 flop/byte);
the big tensors are shipped as fp16 (halves DMA traffic; all arithmetic and
accumulation stay fp32 — absmax error ~1e-3 of output scale).

Per-core device schedule (7 blocks of K<=128 (s,t)-rows):
  DMA   : wr/rz/rc block tiles [K, 80, 80] fp16 (12.8KB contiguous per row)
  DVE   : comb(f32) = wr*rz ; comb += rc ; c2 = reduce_v(comb) -> [K, 80]
  PE    : psum_y[1,80] += layer2_block[K,1].T @ c2[K,80]  (PSUM-accumulated)
"""

import numpy as np

import concourse.bacc as bacc
import concourse.mybir as mybir
import concourse.tile as tile
from concourse.bass_utils import run_bass_kernel_spmd

N = 80
N_CORES = 8
S_PER_CORE = N // N_CORES            # 10
ST = S_PER_CORE * N                  # 800 (s,t) rows per core
N_BLOCKS = 7                         # 6*128 + 32
F32 = mybir.dt.float32
F16 = mybir.dt.float16

_CACHE = {}


def build_nc(repeats=1):
    nc = bacc.Bacc()
    wr_d = nc.declare_dram_parameter("wr", [ST, N, N], F16, isOutput=False)
    rz_d = nc.declare_dram_parameter("rz", [ST, N, N], F16, isOutput=False)
    rc_d = nc.declare_dram_parameter("rc", [ST, N, N], F16, isOutput=False)
    l2_d = nc.declare_dram_parameter("l2", [128, N_BLOCKS], F32, isOutput=False)
    y_d = nc.declare_dram_parameter("y", [1, N], F32, isOutput=True)

    with tile.TileContext(nc) as tc:
        with (
            tc.tile_pool(name="io", bufs=2) as pool,
            tc.tile_pool(name="small", bufs=1) as sp,
            tc.psum_pool(name="ps", bufs=1) as pp,
        ):
            l2_sb = sp.tile([128, N_BLOCKS], F32)
            nc.sync.dma_start(out=l2_sb[:], in_=l2_d[:])
            ypsum = pp.tile([1, N], F32)

            for r in range(repeats):
                for b in range(N_BLOCKS):
                    r0 = b * 128
                    K = min(128, ST - r0)
                    wr_t = pool.tile([128, N, N], F16, tag="wr", bufs=3)
                    rz_t = pool.tile([128, N, N], F16, tag="rz", bufs=3)
                    rc_t = pool.tile([128, N, N], F16, tag="rc", bufs=3)
                    nc.sync.dma_start(out=wr_t[:K], in_=wr_d[r0 : r0 + K])
                    nc.scalar.dma_start(out=rz_t[:K], in_=rz_d[r0 : r0 + K])
                    nc.sync.dma_start(out=rc_t[:K], in_=rc_d[r0 : r0 + K])

                    comb = pool.tile([128, N, N], F32, tag="comb")
                    nc.vector.tensor_mul(out=comb[:K], in0=wr_t[:K], in1=rz_t[:K])
                    nc.vector.tensor_add(out=comb[:K], in0=comb[:K], in1=rc_t[:K])

                    c2 = pool.tile([128, N], F32, tag="c2")
                    nc.vector.tensor_reduce(
                        out=c2[:K],
                        in_=comb[:K],
                        axis=mybir.AxisListType.X,
                        op=mybir.AluOpType.add,
                    )
                    nc.tensor.matmul(
                        ypsum[:],
                        l2_sb[0:K, b : b + 1],
                        c2[:K],
                        start=(b == 0),
                        stop=(b == N_BLOCKS - 1),
                    )

            y_sb = sp.tile([1, N], F32)
            nc.vector.tensor_copy(out=y_sb[:], in_=ypsum[:])
            nc.sync.dma_start(out=y_d[:], in_=y_sb[:])
    nc.compile()
    return nc


def _get_nc():
    if "nc" not in _CACHE:
        _CACHE["nc"] = build_nc()
    return _CACHE["nc"]


def make_in_maps(x, r_zeros, r_const, weights_t, weights_r):
    l2 = (np.asarray(x, np.float32) * np.asarray(weights_t, np.float32))
    wr16 = np.asarray(weights_r, np.float32).astype(np.float16)
    rz16 = np.asarray(r_zeros, np.float32).astype(np.float16)
    rc16 = np.asarray(r_const, np.float32).astype(np.float16)
    in_maps = []
    for c in range(N_CORES):
        sl = slice(c * S_PER_CORE, (c + 1) * S_PER_CORE)
        l2p = np.zeros(128 * N_BLOCKS, np.float32)
        l2p[:ST] = l2[sl].reshape(-1)
        l2cols = np.ascontiguousarray(l2p.reshape(N_BLOCKS, 128).T)
        in_maps.append(
            {
                "wr": wr16[sl].reshape(ST, N, N),
                "rz": rz16[sl].reshape(ST, N, N),
                "rc": rc16[sl].reshape(ST, N, N),
                "l2": l2cols,
            }
        )
    return in_maps


def run(x, r_zeros, r_const, weights_t, weights_r, **spmd_kwargs):
    nc = _get_nc()
    in_maps = make_in_maps(x, r_zeros, r_const, weights_t, weights_r)
    res = run_bass_kernel_spmd(nc, in_maps, list(range(N_CORES)), **spmd_kwargs)
    y = np.zeros(N, np.float32)
    for i in range(N_CORES):
        y += res.results[i]["y"].reshape(N)
    return y, res


def kernel(x, r_zeros, r_const, weights_t, weights_r):
    y, _ = run(x, r_zeros, r_const, weights_t, weights_r)
    return y



# revision 11
# speedup vs baseline: 1.2215x; 1.2215x over previous
"""Trainium2 Bass kernel for nn_DegreePrediction.

Computes y[u] = sum_{s,t,v} (x*W_t)[s,t] * (W_r*r_zeros + r_const)[s,t,u,v]
with N=80, streaming the three rank-4 tensors from HBM as f16.

Sharding: leading s axis split across 8 cores (10 s-values = 800 (s,t) rows
per core, contiguous in DRAM). Each core computes partial accumulations;
host sums the 8 cores' tiny [66,30] partial outputs.

Device schedule per core (st rows on partitions, uv on the free axis):
  DMA  : wr/rz/rc block tiles [K, 6400] f16 on the two HWDGE rings
  DVE  : p16 = wr * rz  (f16 out -> 2x perf mode, ~3.4us per 128-row block)
  PE   : 16 uv-chunks of width 400 (= 5 u-groups of 80 v's);
         ps_chunk[2, 400] += l2hl[:, 2b:2b+2].T @ {rc,p16}_chunk
         accumulated in PSUM across all blocks.  l2 = x*W_t is shipped as
         an f16 (hi, lo) column pair so the contraction carries full f32
         precision of l2.  Chunk c lives in PSUM tile c//3 (6 one-bank
         tiles of [66, 400]) at partition base 32*(c%3) (matmul output
         base-partition must be 0/32/64).
  end  : 6 DVE tensor_reduces fold v on-device: [66, 5, 80] -> [66, 5];
         one [66, 30] f32 DMA out.  Host maps (tile, base, slot) -> u,
         adds the hi+lo rows, sums cores.

Variant "dve2" keeps rc on the DVE (p16 = wr*rz + rc, two 2x passes) and
halves the PE matmul count — fallback in case PE clock gating binds.

Block row-schedule [32, 96, 128x5, 32] plus a free-axis split of the last
block trims NEFF fill/drain; steady state is HBM-DMA-roofline-bound
(~30.7 MB/core f16 at ~360 GB/s).
"""

import numpy as np

import concourse.bacc as bacc
import concourse.mybir as mybir
import concourse.tile as tile
from concourse.bass_utils import run_bass_kernel_spmd

N = 80
N_CORES = 8
S_PER_CORE = N // N_CORES            # 10
ST = S_PER_CORE * N                  # 800 (s,t) rows per core
FREE = N * N                         # 6400 (u,v) columns
BLOCKS = [32, 96, 128, 128, 128, 128, 128, 32]   # sum == ST
NB = len(BLOCKS)
CHUNK = 400                          # 5 u-groups; 16 chunks cover 6400
NCHUNK = FREE // CHUNK               # 16
NPT = (NCHUNK + 2) // 3              # 6 PSUM bank-tiles
UPC = CHUNK // N                     # 5 u's per chunk
F32 = mybir.dt.float32
F16 = mybir.dt.float16

VARIANT = "pe2"                      # "pe2": rc via PE; "dve2": rc via DVE add

_CACHE = {}


def _pieces(b):
    """Free-axis pieces for block b (last block split to shrink drain)."""
    if b == NB - 1:
        return [(0, FREE // 2), (FREE // 2, FREE)]
    return [(0, FREE)]


def _chunks(f0, f1):
    return [(c, c * CHUNK, (c + 1) * CHUNK) for c in range(f0 // CHUNK, f1 // CHUNK)]


def build_nc(repeats=1, variant=None):
    variant = variant or VARIANT
    nc = bacc.Bacc()
    wr_d = nc.declare_dram_parameter("wr", [ST, FREE], F16, isOutput=False)
    rz_d = nc.declare_dram_parameter("rz", [ST, FREE], F16, isOutput=False)
    rc_d = nc.declare_dram_parameter("rc", [ST, FREE], F16, isOutput=False)
    l2_d = nc.declare_dram_parameter("l2", [128, 2 * NB], F16, isOutput=False)
    y_d = nc.declare_dram_parameter("y", [66, NPT * UPC], F32, isOutput=True)

    row0 = np.cumsum([0] + BLOCKS[:-1]).tolist()

    with tile.TileContext(nc) as tc:
        with (
            tc.tile_pool(name="io", bufs=2) as pool,
            tc.tile_pool(name="small", bufs=1) as sp,
            tc.psum_pool(name="ps", bufs=1) as pp,
        ):
            l2_sb = sp.tile([128, 2 * NB], F16)
            nc.sync.dma_start(out=l2_sb[:], in_=l2_d[:])
            pst = [
                pp.tile([66, UPC, N], F32, tag=f"ps{k}", name=f"ps{k}")
                for k in range(NPT)
            ]

            def ps_out(c):
                return pst[c // 3][32 * (c % 3) : 32 * (c % 3) + 2]

            for _ in range(repeats):
                for b in range(NB):
                    r0, K = row0[b], BLOCKS[b]
                    lhsT = l2_sb[0:K, 2 * b : 2 * b + 2]
                    for pi, (f0, f1) in enumerate(_pieces(b)):
                        wr_t = pool.tile([128, FREE], F16, tag="wr", bufs=3)
                        rz_t = pool.tile([128, FREE], F16, tag="rz", bufs=3)
                        rc_t = pool.tile([128, FREE], F16, tag="rc", bufs=3)
                        q0 = nc.sync if (b + pi) % 2 == 0 else nc.scalar
                        q1 = nc.scalar if (b + pi) % 2 == 0 else nc.sync
                        q0.dma_start(out=wr_t[:K, f0:f1], in_=wr_d[r0 : r0 + K, f0:f1])
                        q1.dma_start(out=rz_t[:K, f0:f1], in_=rz_d[r0 : r0 + K, f0:f1])
                        q0.dma_start(out=rc_t[:K, f0:f1], in_=rc_d[r0 : r0 + K, f0:f1])

                        p_t = pool.tile([128, FREE], F16, tag="p", bufs=2)
                        nc.vector.tensor_mul(
                            out=p_t[:K, f0:f1], in0=wr_t[:K, f0:f1], in1=rz_t[:K, f0:f1]
                        )
                        if variant == "dve2":
                            nc.vector.tensor_add(
                                out=p_t[:K, f0:f1], in0=p_t[:K, f0:f1],
                                in1=rc_t[:K, f0:f1],
                            )
                        first, last = (b == 0), (b == NB - 1)
                        if variant == "pe2":
                            for c, a, e in _chunks(f0, f1):
                                nc.tensor.matmul(
                                    ps_out(c), lhsT, rc_t[:K, a:e],
                                    start=first, stop=False,
                                )
                            for c, a, e in _chunks(f0, f1):
                                nc.tensor.matmul(
                                    ps_out(c), lhsT, p_t[:K, a:e],
                                    start=False, stop=last,
                                )
                        else:
                            for c, a, e in _chunks(f0, f1):
                                nc.tensor.matmul(
                                    ps_out(c), lhsT, p_t[:K, a:e],
                                    start=first, stop=last,
                                )

            # fold v on-device: [66, 5, 80] -> [66, 5] per PSUM tile
            y_sb = sp.tile([66, NPT * UPC], F32)
            for k in range(NPT):
                nc.vector.tensor_reduce(
                    out=y_sb[:, k * UPC : (k + 1) * UPC],
                    in_=pst[k][:],
                    axis=mybir.AxisListType.X,
                    op=mybir.AluOpType.add,
                )
            nc.sync.dma_start(out=y_d[:], in_=y_sb[:])
    nc.compile()
    return nc


def _get_nc():
    key = ("nc", VARIANT)
    if key not in _CACHE:
        _CACHE[key] = build_nc()
    return _CACHE[key]


def make_in_maps(x, r_zeros, r_const, weights_t, weights_r):
    l2 = np.asarray(x, np.float32) * np.asarray(weights_t, np.float32)
    wr16 = np.asarray(weights_r, np.float32).astype(np.float16)
    rz16 = np.asarray(r_zeros, np.float32).astype(np.float16)
    rc16 = np.asarray(r_const, np.float32).astype(np.float16)
    row0 = np.cumsum([0] + BLOCKS[:-1]).tolist()
    in_maps = []
    for c in range(N_CORES):
        sl = slice(c * S_PER_CORE, (c + 1) * S_PER_CORE)
        l2c = l2[sl].reshape(ST)
        l2hl = np.zeros((128, 2 * NB), np.float16)
        for b in range(NB):
            r0, K = row0[b], BLOCKS[b]
            seg = l2c[r0 : r0 + K]
            h = seg.astype(np.float16)
            l2hl[:K, 2 * b] = h
            l2hl[:K, 2 * b + 1] = (seg - h.astype(np.float32)).astype(np.float16)
        in_maps.append(
            {
                "wr": wr16[sl].reshape(ST, FREE),
                "rz": rz16[sl].reshape(ST, FREE),
                "rc": rc16[sl].reshape(ST, FREE),
                "l2": l2hl,
            }
        )
    return in_maps


def run(x, r_zeros, r_const, weights_t, weights_r, **spmd_kwargs):
    nc = _get_nc()
    in_maps = make_in_maps(x, r_zeros, r_const, weights_t, weights_r)
    res = run_bass_kernel_spmd(nc, in_maps, list(range(N_CORES)), **spmd_kwargs)
    y = np.zeros(N, np.float64)
    for i in range(N_CORES):
        P = res.results[i]["y"].astype(np.float64)       # [66, NPT*UPC]
        for c in range(NCHUNK):
            k, m = c // 3, c % 3
            cols = slice(k * UPC, k * UPC + UPC)
            # hi row + lo row of chunk c -> y[5c : 5c+5]
            y[UPC * c : UPC * c + UPC] += P[32 * m, cols] + P[32 * m + 1, cols]
    return y.astype(np.float32), res


def kernel(x, r_zeros, r_const, weights_t, weights_r):
    y, _ = run(x, r_zeros, r_const, weights_t, weights_r)
    return y
